# revision 44
# baseline (speedup 1.0000x reference)
"""Trainium2 Bass kernel for a 3-layer FCL + size-5 sliding-window stack.

Reference computation (fp32):
    h = relu(x @ W1.T)          # [N, 10]
    t = relu(h @ W2.T + b2)     # [N, 5]
    out[n] = concat(t[n-2..n+2])  zero-padded  -> [N, 25]

Strategy (8 cores, data-parallel over rows, halo recomputed per core).
The kernel is HBM-stream-bound: per core it reads 8MB of fp8 x and
writes 0.67MB of fp8 windowed output, and the whole schedule is built
around keeping ONE DMA queue saturated at the ~420GB/s wall while the
PE consumes groups just behind the stream.

  - Precision: x is cast to fp8-e3m4 on host (quarter the HBM read of
    fp32; e4m3 - which would enable DoubleRow 2x matmul - measures
    2.3e-2 end-to-end, over the 2e-2 gate).  Weights are bf16
    stationary (fp8e3 moving runs at bf16 speed, fp32 PSUM accum).
    The output t is stored as fp8-e3m4 scaled by x8 (dodges the e3m4
    subnormal floor below 0.25; host divides by 8): rel err 1.78e-2.
  - Compute is organized in GROUPS OF FOUR 512-col blocks whose h
    accumulators live at partition strips {0-9,32-41,64-73,96-105} of
    ONE psum bank via tile_position col strips.  Matmuls on different
    32-col sub-array quadrants run CONCURRENTLY (separate xbus streams
    + per-quadrant weights), so a group's 10 L1 matmuls + 1 stacked L2
    matmul take ~1.25us warm - ~4x the baseline's throughput:
      * c3 (the 64-row tail of K=320=128+128+64) is PACKED: two blocks'
        tails ride one full-height matmul with a block-diagonal
        [128, 64] weight.  These run FIRST with start=True, writing the
        strips' full 2KB PSUM zero region so later accumulates land on
        defined values.  c1/c2 accumulate with one LDW per quadrant,
        hidden under other quadrants' matmuls.
      * L2 is ONE matmul per group: block-diagonal W2 [128, 100] (rows
        32i+r -> cols 25i+m = W2rep, the x5 window-replicated W2.T).
      * DVE: one [128,512] psum->sbuf bf16 relu per group.  ACT: one
        [100,512] bias+relu (bias per-partition, x8 scale) per group.
  - DMA model (measured): throughput ~= 16 engines x packet / ~250ns,
    where a packet is one per-partition contiguous run; queues split
    the engine pool when active concurrently.  Consequences baked in:
      * x is loaded as SEVEN need-ordered region DMAs (2 groups each,
        10KB per-partition runs) ALL ON THE SP RING: serial delivery
        means the earliest-needed region gets the full ~420GB/s.
      * the host pre-packs x into those regions (XP): per partition
        [c1 2048 | c2 2048 | paired-c3 1024] per group, with the c3
        block pair halves pre-interleaved to partition halves, fully
        contiguous in dram per region.  Host prep is layout+cast only.
      * weights (packed [128,184] bf16) load AFTER region 0 on the same
        queue: their 368B packets would otherwise straggle the region
        packets via the shared engines.  b2 arrives pre-replicated to
        100 partitions (a stride-0 broadcast DMA emits 100 4-byte
        packets that clog the queue for ~10us - measured).
      * each region tile gets its OWN tile pool: consecutive tiles of
        one pool chain their writer DMAs (measured +10us serialization).
  - The size-5 window gather costs nothing on device: tT_g holds t.T
    x5-replicated as [100 partitions = 25i+5w+c, 13 groups x 512] fp8,
    and each store wave is ONE 2-dim DMA (partitions enumerate (i,r)
    i-major = outG's plane order).  Stores are SBUF-read-port bound
    (~6.7KB/partition total), so they run in 3 late waves on the
    scalar/gpsimd rings, overlapping the compute tail.  The host
    de-tiles the block grouping and applies the w-shift slices - pure
    layout, like the baseline's transpose.
  - HAM warmup: 9 full-width matmuls on scratch lift the PE clock to
    2.4GHz during the initial region-0 fill.  Total PE busy is ~16us,
    far below the power-manager derating budget (the baseline spent its
    last 35us throttled to 1.2GHz).
  - Host unshard: de-group outG, shift-slice per window w, upcast /8,
    concat cores, transpose, patch the 4 global-edge window slots to
    exact zero (the reference zero-pads t, not x).
  - The ISA allows ONE sync-wait per instruction; a post-pass hoists
    any extra waits onto same-engine NoOps.

Measured: 121.7us (original baseline) -> 39-42us, rel err 1.78e-2.
"""

import numpy as np
import ml_dtypes

import bass_rust
import concourse.bass as bass
import concourse.mybir as mybir
import concourse.tile as tile

# ---- problem constants (hardcoded per contract) ----
N = 200000
D = 320
D1 = 10
D2 = 5
W = 5
HALF = W // 2
NCORES = 8
ROWS = N // NCORES          # 25000 output rows per core
BLK = 512                   # t-cols per block (one PSUM bank)
NBLK = 49                   # 25088 padded t-cols per core
PAD = NBLK * BLK            # 25088
GRP = 4                     # blocks per group (4 psum strips)
NGRP = 13                   # 12 full groups + 1 single-block group
NWARM = 9                   # warmup matmuls (~4.3us cold) lift HAM to 2.4GHz
F32 = mybir.dt.float32
BF16 = mybir.dt.bfloat16
FP8 = mybir.dt.float8e3
RELU = mybir.ActivationFunctionType.Relu
BF = ml_dtypes.bfloat16
F8 = ml_dtypes.float8_e3m4

# wc_sb column layout: [c1 0:10 | c2 10:20 | c3 pair-diag 20:84 | W2 blockdiag 84:184]
WC1, WC2, WC3, WL2, WCEND = 0, 10, 20, 84, 184

# load regions: groups [g0, g1), per-partition width (5120 per full
# group: [c1 2048 | c2 2048 | paired-c3 1024]; g12 is 1536 tight).
# Pairs of groups -> 10KB packets (~420GB/s), one region per DMA so the
# completion semaphore releases compute promptly; regions round-robin
# the 3 rings so no region's semaphore queues behind another's packets.
REGIONS = [(0, 1, 5120), (1, 3, 10240), (3, 5, 10240), (5, 7, 10240),
           (7, 9, 10240), (9, 10, 5120), (10, 11, 5120), (11, 12, 5120),
           (12, 13, 1536)]
REGION_RING = [0] * 9
XPTOT = sum(w for _, _, w in REGIONS)

_NC_CACHE = {}


def split_multiwaits(nc):
    """Walrus/ISA allows ONE sync-wait per instruction; Tile emits several.

    For every instruction with >1 wait, hoist all but the last wait onto
    fresh NoOps on the same engine immediately before it.  The engine
    stalls at the nops exactly as it would have at the instruction, so
    semantics are unchanged.
    """
    n_split = 0
    for bb in nc.main_func.blocks:
        insts = bb.instructions
        out = []
        changed = False
        for ins in insts:
            si = ins.sync_info
            waits = list(si.on_wait) if si is not None else []
            if len(waits) > 1:
                changed = True
                for w in waits[:-1]:
                    n_split += 1
                    nop = bass_rust.InstNoOp(name=f"wsplit-{n_split}")
                    nop.engine = ins.engine
                    nop.sync_info = bass_rust.SyncInfo(
                        on_wait=[w], on_update=[]
                    )
                    nc.inst_map[nop.name] = nop
                    out.append(nop)
                ins.sync_info = bass_rust.SyncInfo(
                    on_wait=[waits[-1]], on_update=list(si.on_update)
                )
            out.append(ins)
        if changed:
            bb.instructions = out
    return n_split


def build_nc():
    nc = bass.Bass("TRN2", target_bir_lowering=False, debug=False)

    # XP: per-superblock contiguous regions so every load DMA is one
    # fully-contiguous dram read: [chunk1 p-major 512KB | chunk2 512KB |
    # paired-chunk3 256KB] per superblock
    # need-ordered load regions (all on one queue; DMA throughput is
    # ~16 engines x packet/250ns, so 10-20KB per-partition runs reach
    # the ~390GB/s HBM wall on a single queue)
    xp_t = nc.dram_tensor("XP", [XPTOT * 128], FP8, kind="ExternalInput")
    wc_t = nc.dram_tensor("WC", [128, WCEND], BF16, kind="ExternalInput")
    b2r_t = nc.dram_tensor("B2R", [100], F32, kind="ExternalInput")
    # grouped output: outG[i, 5w+c, 512g+jj] = t[c, 2048g+512i+jj]
    # (host de-tiles the block grouping and applies the w-shift slices)
    # fp8-e3m4 output with a x8 pre-scale (dodges the e3m4 subnormal
    # floor below 0.25): halves the SBUF-port-bound store drain.
    # Host divides by 8; measured end-to-end rel err 1.78e-2 (gate 2e-2).
    outG_t = nc.dram_tensor(
        "outG", [GRP, W * D2, NGRP * BLK], FP8, kind="ExternalOutput"
    )

    with tile.TileContext(nc) as tc:
        with (
            tc.tile_pool(name="singles", bufs=1) as singles,
            tc.tile_pool(name="xr0", bufs=1) as xr0p,
            tc.tile_pool(name="xr1", bufs=1) as xr1p,
            tc.tile_pool(name="xr2", bufs=1) as xr2p,
            tc.tile_pool(name="xr3", bufs=1) as xr3p,
            tc.tile_pool(name="xr4", bufs=1) as xr4p,
            tc.tile_pool(name="xr5", bufs=1) as xr5p,
            tc.tile_pool(name="xr6", bufs=1) as xr6p,
            tc.tile_pool(name="xr7", bufs=1) as xr7p,
            tc.tile_pool(name="xr8", bufs=1) as xr8p,
            tc.tile_pool(name="hspool", bufs=3) as hspool,
            tc.tile_pool(name="ps_h", bufs=3, space="PSUM") as ps_h,
            tc.tile_pool(name="ps_t", bufs=2, space="PSUM") as ps_t,
            tc.tile_pool(name="ps_w", bufs=1, space="PSUM") as ps_w,
        ):
            # ---- R0 heads the load queue; the constants' tiny packets
            # (368B x 128) would delay it ~2.5us. wc lands before R1 so
            # the first LDWEIGHTS (~R0-arrival) never waits. ----
            RINGS = [nc.sync, nc.scalar, nc.gpsimd]
            xr_pools = [xr0p, xr1p, xr2p, xr3p, xr4p, xr5p, xr6p, xr7p, xr8p]
            xr_tiles = []
            reg_offs = []
            off = 0
            for g0, g1, wdt in REGIONS:
                reg_offs.append(off)
                off += wdt
            def emit_region(ri):
                g0, g1, wdt = REGIONS[ri]
                xr = xr_pools[ri].tile([128, wdt], FP8)
                RINGS[REGION_RING[ri]].dma_start(
                    out=xr,
                    in_=bass.AP(
                        xp_t, reg_offs[ri] * 128, [[wdt, 128], [1, wdt]]
                    ),
                )
                xr_tiles.append(xr)
            emit_region(0)
            wc_sb = singles.tile([128, WCEND], BF16)
            nc.sync.dma_start(out=wc_sb, in_=wc_t[:, :])
            # b2 pre-replicated by the host: b2r[25i+5w+c] = b2[c]
            b2r_sb = singles.tile([100, 1], F32)
            nc.sync.dma_start(
                out=b2r_sb, in_=bass.AP(b2r_t, 0, [[1, 100], [1, 1]])
            )
            for ri in range(1, len(REGIONS)):
                emit_region(ri)
            reg_of = {}
            for ri, (g0, g1, wdt) in enumerate(REGIONS):
                for g in range(g0, g1):
                    reg_of[g] = (ri, g - g0)

            # persistent grouped t.T accumulator [100, 13, 512] fp8 (x8)
            tT_g = singles.tile([100, NGRP, BLK], FP8)

            # ---- HAM warmup: full-width matmuls on scratch while the
            # first x loads stream in (PE is otherwise idle).  The HAM
            # window needs >=3.4us of sustained activity; 7 cold N=512
            # matmuls are ~4.3us. ----
            warm_sb = singles.tile([128, BLK], BF16)
            nc.vector.memset(warm_sb, 0.625)
            warm_ps = ps_w.tile([128, BLK], F32, tag="w")
            for i in range(NWARM):
                nc.tensor.matmul(
                    warm_ps, warm_sb[:, :128], warm_sb,
                    start=True, stop=True,
                )

            h_pss = {}      # group -> h psum tile [128, 512]
            hs_sbs = {}     # group -> relu'd h [128, 512] bf16
            t_pss = {}      # group -> tT psum tile [100, 512]

            def emit_group_mms(g):
                """10 matmuls for the 4 blocks of group g, strip-rotated."""
                nb = GRP if g < NGRP - 1 else 1
                ri, k = reg_of[g]
                xg = xr_tiles[ri]
                co1, co2, co3 = (
                    (5120 * k, 5120 * k + 2048, 5120 * k + 4096)
                    if nb == GRP else (0, BLK, 2 * BLK)
                )
                h_ps = ps_h.tile([128, BLK], F32, tag="h")
                # c3 pair matmuls first: start=True writes the strips'
                # full 2KB zero region (zeros where the diag weight is 0)
                for p in range(2 if nb == GRP else 1):
                    nc.tensor.matmul(
                        h_ps[64 * p : 64 * p + 64, :],
                        wc_sb[:, WC3:WL2],
                        xg[:, co3 + BLK * p : co3 + BLK * (p + 1)],
                        start=True, stop=False,
                        skip_group_check=True,
                        tile_position=(0, 64 * p),
                    )
                for co, w0 in ((co1, WC1), (co2, WC2)):
                    last = w0 == WC2
                    for i in range(nb):
                        nc.tensor.matmul(
                            h_ps[32 * i : 32 * i + D1, :],
                            wc_sb[:, w0 : w0 + D1],
                            xg[:, co + BLK * i : co + BLK * (i + 1)],
                            start=False, stop=last,
                            skip_group_check=True,
                            tile_position=(0, 32 * i),
                        )
                h_pss[g] = h_ps

            def emit_relu(g):
                """DVE: one relu+cast for the whole group's h strips."""
                nparts = 128 if g < NGRP - 1 else 42
                hs = hspool.tile([128, BLK], BF16, tag="hs")
                nc.vector.tensor_scalar_max(
                    hs[:nparts, :], h_pss[g][:nparts, :], 0.0
                )
                hs_sbs[g] = hs
                del h_pss[g]

            def emit_l2(g):
                """One stacked L2 matmul: block-diag W2 [128,100] @ h."""
                nk = 128 if g < NGRP - 1 else 42
                t_ps = ps_t.tile([100, BLK], F32, tag="t")
                nc.tensor.matmul(
                    t_ps, wc_sb[:nk, WL2:WCEND], hs_sbs[g][:nk, :],
                    start=True, stop=True,
                )
                t_pss[g] = t_ps
                del hs_sbs[g]

            def emit_act(g):
                """ACT: tT_g[:, g, :] = relu(t_ps + b2r)."""
                nc.scalar.activation(
                    tT_g[:, g, :],
                    t_pss[g],
                    RELU,
                    bias=b2r_sb,
                    scale=8.0,
                )
                del t_pss[g]

            def emit_store(g0, g1, r0):
                """Store tT_g groups [g0, g1) to dram in the grouped
                layout with ONE 2-dim DMA: src partitions 0-99 enumerate
                (i, 5w+c) i-major, exactly outG's [i, r] plane order.
                Host de-tiles the grouping.  Scalar/gpsimd rings only -
                sync is the load artery."""
                RINGS[1 + r0 % 2].dma_start(
                    out=bass.AP(
                        outG_t,
                        BLK * g0,
                        [[NGRP * BLK, 100], [1, BLK * (g1 - g0)]],
                    ),
                    in_=tT_g[:, g0:g1, :],
                )

            # ---- main loop (software-pipelined, one iteration per group) ----
            for g in range(NGRP):
                emit_group_mms(g)
                if g >= 1:
                    emit_l2(g - 1)
                emit_relu(g)
                if g >= 1:
                    emit_act(g - 1)
                if g == 11:
                    emit_store(0, 9, 0)
                elif g == 12:
                    emit_store(9, 12, 1)
            emit_l2(NGRP - 1)
            emit_act(NGRP - 1)
            emit_store(NGRP - 1, NGRP, 1)

    split_multiwaits(nc)
    return nc


def make_shards(x):
    """Per-core xT [320, PAD] fp8-e3m4 shards, +-2 col halo, zero padded."""
    xbT = np.ascontiguousarray(x.astype(F8).T)  # [320, N]
    shards = []
    for c in range(NCORES):
        s = np.zeros((D, PAD), dtype=F8)
        lo = ROWS * c - HALF
        src_lo, src_hi = max(lo, 0), min(lo + PAD, N)
        s[:, src_lo - lo : src_lo - lo + (src_hi - src_lo)] = xbT[
            :, src_lo:src_hi
        ]
        shards.append(s)
    return shards


def make_xp(xbT):
    """Need-ordered flat load regions from one core's xT [320, PAD]."""
    out = []
    for g0, g1, wdt in REGIONS:
        reg = np.zeros((128, wdt), dtype=F8)
        for g in range(g0, g1):
            ncols = 2048 if g < NGRP - 1 else BLK
            cs = 2048 * g
            k = g - g0
            c0 = 5120 * k if g < NGRP - 1 else 0
            step = 2048 if g < NGRP - 1 else BLK
            reg[:, c0 : c0 + ncols] = xbT[0:128, cs : cs + ncols]
            reg[:, c0 + step : c0 + step + ncols] = xbT[128:256, cs : cs + ncols]
            c3 = xbT[256:320, cs : cs + ncols]
            if g < NGRP - 1:
                reg[:, c0 + 4096 : c0 + 5120] = (
                    c3.reshape(64, 2, 2, BLK).transpose(2, 0, 1, 3).reshape(128, 1024)
                )
            else:
                reg[0:64, c0 + 2 * BLK : c0 + 3 * BLK] = c3
        out.append(reg)
    return np.concatenate([r.reshape(-1) for r in out])


def make_wc(W1, W2):
    """Packed bf16 stationary weights [128, 184]."""
    wc = np.zeros((128, WCEND), dtype=np.float32)
    W1T = W1.T  # [320, 10]
    wc[:, WC1:WC1 + D1] = W1T[0:128]
    wc[:, WC2:WC2 + D1] = W1T[128:256]
    wc[0:64, WC3:WC3 + D1] = W1T[256:320]
    wc[64:128, WC3 + 32 : WC3 + 32 + D1] = W1T[256:320]
    W2rep = np.tile(W2.T, (1, W))  # [10, 25]
    for i in range(4):
        wc[32 * i : 32 * i + D1, WL2 + 25 * i : WL2 + 25 * (i + 1)] = W2rep
    return np.ascontiguousarray(wc.astype(BF))


def _patch_edges(out):
    # the reference zero-pads t, not x: window slots that fall outside
    # [0, N) must be exactly zero.
    out[0, : 2 * D2] = 0.0
    out[1, :D2] = 0.0
    out[N - 2, 4 * D2 :] = 0.0
    out[N - 1, 3 * D2 :] = 0.0
    return out


def run(inputs, trace=False):
    from concourse.bass_utils import run_bass_kernel_spmd

    x = np.ascontiguousarray(np.asarray(inputs["x"], dtype=np.float32))
    W1 = np.asarray(inputs["W1"], dtype=np.float32)
    W2 = np.asarray(inputs["W2"], dtype=np.float32)
    b2 = np.ascontiguousarray(np.asarray(inputs["b2"], dtype=np.float32))
    assert x.shape == (N, D)

    WC = make_wc(W1, W2)

    if "nc" not in _NC_CACHE:
        _NC_CACHE["nc"] = build_nc()
    nc = _NC_CACHE["nc"]

    B2R = np.ascontiguousarray(np.tile(b2 * 8.0, 20).astype(np.float32))
    in_maps = [
        {"XP": make_xp(s), "WC": WC, "B2R": B2R} for s in make_shards(x)
    ]
    res = run_bass_kernel_spmd(nc, in_maps, list(range(NCORES)), trace=trace)
    cores = []
    for c in range(NCORES):
        og = np.asarray(res.results[c]["outG"]).astype(np.float32) / 8.0
        # de-tile the block grouping: [i, r, 512g+jj] -> [r, 2048g+512i+jj]
        flat = np.ascontiguousarray(
            og.reshape(GRP, 25, NGRP, BLK).transpose(1, 2, 0, 3)
        ).reshape(25, GRP * NGRP * BLK)
        core = np.empty((25, ROWS), dtype=np.float32)
        for w in range(W):  # out[5w+c, n] = t[c, n+w] = flat[5w+c, n+w]
            core[5 * w : 5 * w + D2] = flat[5 * w : 5 * w + D2, w : w + ROWS]
        cores.append(core)
    out = np.ascontiguousarray(np.concatenate(cores, axis=1).T)
    return _patch_edges(out), res


def kernel(**inputs):
    out, _ = run(inputs, trace=False)
    return out


# revision 46
# speedup vs baseline: 1.0509x; 1.0509x over previous
"""Trainium2 Bass kernel for a 3-layer FCL + size-5 sliding-window stack.

Reference computation (fp32):
    h = relu(x @ W1.T)          # [N, 10]
    t = relu(h @ W2.T + b2)     # [N, 5]
    out[n] = concat(t[n-2..n+2])  zero-padded  -> [N, 25]

Strategy (8 cores, data-parallel over rows, halo recomputed per core).
The kernel is HBM-stream-bound: per core it reads 8MB of fp8 x and
writes 0.67MB of fp8 windowed output, and the whole schedule is built
around keeping ONE DMA queue saturated at the ~420GB/s wall while the
PE consumes groups just behind the stream.

  - Precision: x is cast to fp8-e3m4 on host (quarter the HBM read of
    fp32; e4m3 - which would enable DoubleRow 2x matmul - measures
    2.3e-2 end-to-end, over the 2e-2 gate).  Weights are bf16
    stationary (fp8e3 moving runs at bf16 speed, fp32 PSUM accum).
    The output t is stored as fp8-e3m4 scaled by x8 (dodges the e3m4
    subnormal floor below 0.25; host divides by 8): rel err 1.78e-2.
  - Compute is organized in GROUPS OF FOUR 512-col blocks whose h
    accumulators live at partition strips {0-9,32-41,64-73,96-105} of
    ONE psum bank via tile_position col strips.  Matmuls on different
    32-col sub-array quadrants run CONCURRENTLY (separate xbus streams
    + per-quadrant weights), so a group's 10 L1 matmuls + 1 stacked L2
    matmul take ~1.25us warm - ~4x the baseline's throughput:
      * c3 (the 64-row tail of K=320=128+128+64) is PACKED: two blocks'
        tails ride one full-height matmul with a block-diagonal
        [128, 64] weight.  These run FIRST with start=True, writing the
        strips' full 2KB PSUM zero region so later accumulates land on
        defined values.  c1/c2 accumulate with one LDW per quadrant,
        hidden under other quadrants' matmuls.
      * L2 is ONE matmul per group: block-diagonal W2 [128, 100] (rows
        32i+r -> cols 25i+m = W2rep, the x5 window-replicated W2.T).
      * DVE: one [128,512] psum->sbuf bf16 relu per group.  ACT: one
        [100,512] bias+relu (bias per-partition, x8 scale) per group.
  - DMA model (measured): throughput ~= 16 engines x packet / ~250ns,
    where a packet is one per-partition contiguous run; queues split
    the engine pool when active concurrently.  Consequences baked in:
      * x is loaded as NINE need-ordered region DMAs ALL ON THE SP
        RING (single-group head and tail regions for latency, 2-group
        middle regions for 10KB packets): serial delivery means the
        earliest-needed region gets the full ~420GB/s.
      * the host pre-packs x into those regions (XP): per partition
        [c1 2048 | c2 2048 | paired-c3 1024] per group, with the c3
        block pair halves pre-interleaved to partition halves, fully
        contiguous in dram per region.  Host prep is layout+cast only.
      * weights (packed [128,184] bf16) load AFTER region 0 on the same
        queue: their 368B packets would otherwise straggle the region
        packets via the shared engines.  b2 arrives pre-replicated to
        100 partitions (a stride-0 broadcast DMA emits 100 4-byte
        packets that clog the queue for ~10us - measured).
      * each region tile gets its OWN tile pool: consecutive tiles of
        one pool chain their writer DMAs (measured +10us serialization).
  - The size-5 window gather costs nothing on device: tT_g holds t.T
    x5-replicated as [100 partitions = 25i+5w+c, 13 groups x 512] fp8,
    and each store wave is ONE 2-dim DMA (partitions enumerate (i,r)
    i-major = outG's plane order).  Stores are SBUF-read-port bound
    (~6.7KB/partition total), so they run in 3 late waves on the
    scalar/gpsimd rings, overlapping the compute tail.  The host
    de-tiles the block grouping and applies the w-shift slices - pure
    layout, like the baseline's transpose.
  - HAM warmup: 9 full-width matmuls on scratch lift the PE clock to
    2.4GHz during the initial region-0 fill.  Total PE busy is ~16us,
    far below the power-manager derating budget (the baseline spent its
    last 35us throttled to 1.2GHz).
  - Host unshard: de-group outG, shift-slice per window w, upcast /8,
    concat cores, transpose, patch the 4 global-edge window slots to
    exact zero (the reference zero-pads t, not x).
  - The ISA allows ONE sync-wait per instruction; a post-pass hoists
    any extra waits onto same-engine NoOps.

Measured: 121.7us (original baseline) -> ~40-43us, rel err 1.78e-2.
"""

import numpy as np
import ml_dtypes

import bass_rust
import concourse.bass as bass
import concourse.mybir as mybir
import concourse.tile as tile

# ---- problem constants (hardcoded per contract) ----
N = 200000
D = 320
D1 = 10
D2 = 5
W = 5
HALF = W // 2
NCORES = 8
ROWS = N // NCORES          # 25000 output rows per core
BLK = 512                   # t-cols per block (one PSUM bank)
NBLK = 49                   # 25088 padded t-cols per core
PAD = NBLK * BLK            # 25088
GRP = 4                     # blocks per group (4 psum strips)
NGRP = 13                   # 12 full groups + 1 single-block group
NWARM = 7                   # warmup matmuls (~4.3us cold) lift HAM to 2.4GHz
F32 = mybir.dt.float32
BF16 = mybir.dt.bfloat16
FP8 = mybir.dt.float8e3
RELU = mybir.ActivationFunctionType.Relu
BF = ml_dtypes.bfloat16
F8 = ml_dtypes.float8_e3m4

# wc_sb column layout: [c1 0:10 | c2 10:20 | c3 pair-diag 20:84 | W2 blockdiag 84:184]
WC1, WC2, WC3, WL2, WCEND = 0, 10, 20, 84, 184

# load regions: groups [g0, g1), per-partition width (5120 per full
# group: [c1 2048 | c2 2048 | paired-c3 1024]; g12 is 1536 tight).
# One region per DMA so its completion semaphore releases compute
# promptly; middle regions are pairs (10KB packets ~ 420GB/s), head and
# tail regions are single groups (earlier first compute, incremental
# tail arrivals).  All serial on the SP ring, in need order.
REGIONS = [(0, 1, 5120), (1, 3, 10240), (3, 5, 10240), (5, 7, 10240),
           (7, 9, 10240), (9, 10, 5120), (10, 11, 5120), (11, 12, 5120),
           (12, 13, 1536)]
REGION_RING = [0] * 9
XPTOT = sum(w for _, _, w in REGIONS)

_NC_CACHE = {}


def split_multiwaits(nc):
    """Walrus/ISA allows ONE sync-wait per instruction; Tile emits several.

    For every instruction with >1 wait, hoist all but the last wait onto
    fresh NoOps on the same engine immediately before it.  The engine
    stalls at the nops exactly as it would have at the instruction, so
    semantics are unchanged.
    """
    n_split = 0
    for bb in nc.main_func.blocks:
        insts = bb.instructions
        out = []
        changed = False
        for ins in insts:
            si = ins.sync_info
            waits = list(si.on_wait) if si is not None else []
            if len(waits) > 1:
                changed = True
                for w in waits[:-1]:
                    n_split += 1
                    nop = bass_rust.InstNoOp(name=f"wsplit-{n_split}")
                    nop.engine = ins.engine
                    nop.sync_info = bass_rust.SyncInfo(
                        on_wait=[w], on_update=[]
                    )
                    nc.inst_map[nop.name] = nop
                    out.append(nop)
                ins.sync_info = bass_rust.SyncInfo(
                    on_wait=[waits[-1]], on_update=list(si.on_update)
                )
            out.append(ins)
        if changed:
            bb.instructions = out
    return n_split


def build_nc():
    nc = bass.Bass("TRN2", target_bir_lowering=False, debug=False)

    # XP: per-superblock contiguous regions so every load DMA is one
    # fully-contiguous dram read: [chunk1 p-major 512KB | chunk2 512KB |
    # paired-chunk3 256KB] per superblock
    # need-ordered load regions (all on one queue; DMA throughput is
    # ~16 engines x packet/250ns, so 10-20KB per-partition runs reach
    # the ~390GB/s HBM wall on a single queue)
    xp_t = nc.dram_tensor("XP", [XPTOT * 128], FP8, kind="ExternalInput")
    wc_t = nc.dram_tensor("WC", [128, WCEND], BF16, kind="ExternalInput")
    b2r_t = nc.dram_tensor("B2R", [100], F32, kind="ExternalInput")
    # grouped output: outG[i, 5w+c, 512g+jj] = t[c, 2048g+512i+jj]
    # (host de-tiles the block grouping and applies the w-shift slices)
    # fp8-e3m4 output with a x8 pre-scale (dodges the e3m4 subnormal
    # floor below 0.25): halves the SBUF-port-bound store drain.
    # Host divides by 8; measured end-to-end rel err 1.78e-2 (gate 2e-2).
    outG_t = nc.dram_tensor(
        "outG", [GRP, W * D2, NGRP * BLK], FP8, kind="ExternalOutput"
    )

    with tile.TileContext(nc) as tc:
        with (
            tc.tile_pool(name="singles", bufs=1) as singles,
            tc.tile_pool(name="xr0", bufs=1) as xr0p,
            tc.tile_pool(name="xr1", bufs=1) as xr1p,
            tc.tile_pool(name="xr2", bufs=1) as xr2p,
            tc.tile_pool(name="xr3", bufs=1) as xr3p,
            tc.tile_pool(name="xr4", bufs=1) as xr4p,
            tc.tile_pool(name="xr5", bufs=1) as xr5p,
            tc.tile_pool(name="xr6", bufs=1) as xr6p,
            tc.tile_pool(name="xr7", bufs=1) as xr7p,
            tc.tile_pool(name="xr8", bufs=1) as xr8p,
            tc.tile_pool(name="hspool", bufs=3) as hspool,
            tc.tile_pool(name="ps_h", bufs=3, space="PSUM") as ps_h,
            tc.tile_pool(name="ps_t", bufs=2, space="PSUM") as ps_t,
            tc.tile_pool(name="ps_w", bufs=1, space="PSUM") as ps_w,
        ):
            # ---- R0 heads the load queue; the constants' tiny packets
            # (368B x 128) would delay it ~2.5us. wc lands before R1 so
            # the first LDWEIGHTS (~R0-arrival) never waits. ----
            RINGS = [nc.sync, nc.scalar, nc.gpsimd]
            xr_pools = [xr0p, xr1p, xr2p, xr3p, xr4p, xr5p, xr6p, xr7p, xr8p]
            xr_tiles = []
            reg_offs = []
            off = 0
            for g0, g1, wdt in REGIONS:
                reg_offs.append(off)
                off += wdt
            def emit_region(ri):
                g0, g1, wdt = REGIONS[ri]
                xr = xr_pools[ri].tile([128, wdt], FP8)
                RINGS[REGION_RING[ri]].dma_start(
                    out=xr,
                    in_=bass.AP(
                        xp_t, reg_offs[ri] * 128, [[wdt, 128], [1, wdt]]
                    ),
                )
                xr_tiles.append(xr)
            emit_region(0)
            wc_sb = singles.tile([128, WCEND], BF16)
            nc.sync.dma_start(out=wc_sb, in_=wc_t[:, :])
            # b2 pre-replicated by the host: b2r[25i+5w+c] = b2[c]
            b2r_sb = singles.tile([100, 1], F32)
            nc.sync.dma_start(
                out=b2r_sb, in_=bass.AP(b2r_t, 0, [[1, 100], [1, 1]])
            )
            for ri in range(1, len(REGIONS)):
                emit_region(ri)
            reg_of = {}
            for ri, (g0, g1, wdt) in enumerate(REGIONS):
                for g in range(g0, g1):
                    reg_of[g] = (ri, g - g0)

            # persistent grouped t.T accumulator [100, 13, 512] fp8 (x8)
            tT_g = singles.tile([100, NGRP, BLK], FP8)

            # ---- HAM warmup: full-width matmuls on scratch while the
            # first x loads stream in (PE is otherwise idle).  The HAM
            # window needs >=3.4us of sustained activity; 7 cold N=512
            # matmuls are ~4.3us. ----
            warm_sb = singles.tile([128, BLK], BF16)
            nc.vector.memset(warm_sb, 0.625)
            warm_ps = ps_w.tile([128, BLK], F32, tag="w")
            for i in range(NWARM):
                nc.tensor.matmul(
                    warm_ps, warm_sb[:, :128], warm_sb,
                    start=True, stop=True,
                )

            h_pss = {}      # group -> h psum tile [128, 512]
            hs_sbs = {}     # group -> relu'd h [128, 512] bf16
            t_pss = {}      # group -> tT psum tile [100, 512]

            def emit_group_mms(g):
                """10 matmuls for the 4 blocks of group g, strip-rotated."""
                nb = GRP if g < NGRP - 1 else 1
                ri, k = reg_of[g]
                xg = xr_tiles[ri]
                co1, co2, co3 = (
                    (5120 * k, 5120 * k + 2048, 5120 * k + 4096)
                    if nb == GRP else (0, BLK, 2 * BLK)
                )
                h_ps = ps_h.tile([128, BLK], F32, tag="h")
                # c3 pair matmuls first: start=True writes the strips'
                # full 2KB zero region (zeros where the diag weight is 0)
                for p in range(2 if nb == GRP else 1):
                    nc.tensor.matmul(
                        h_ps[64 * p : 64 * p + 64, :],
                        wc_sb[:, WC3:WL2],
                        xg[:, co3 + BLK * p : co3 + BLK * (p + 1)],
                        start=True, stop=False,
                        skip_group_check=True,
                        tile_position=(0, 64 * p),
                    )
                for co, w0 in ((co1, WC1), (co2, WC2)):
                    last = w0 == WC2
                    for i in range(nb):
                        nc.tensor.matmul(
                            h_ps[32 * i : 32 * i + D1, :],
                            wc_sb[:, w0 : w0 + D1],
                            xg[:, co + BLK * i : co + BLK * (i + 1)],
                            start=False, stop=last,
                            skip_group_check=True,
                            tile_position=(0, 32 * i),
                        )
                h_pss[g] = h_ps

            def emit_relu(g):
                """DVE: one relu+cast for the whole group's h strips."""
                nparts = 128 if g < NGRP - 1 else 42
                hs = hspool.tile([128, BLK], BF16, tag="hs")
                nc.vector.tensor_scalar_max(
                    hs[:nparts, :], h_pss[g][:nparts, :], 0.0
                )
                hs_sbs[g] = hs
                del h_pss[g]

            def emit_l2(g):
                """One stacked L2 matmul: block-diag W2 [128,100] @ h."""
                nk = 128 if g < NGRP - 1 else 42
                t_ps = ps_t.tile([100, BLK], F32, tag="t")
                nc.tensor.matmul(
                    t_ps, wc_sb[:nk, WL2:WCEND], hs_sbs[g][:nk, :],
                    start=True, stop=True,
                )
                t_pss[g] = t_ps
                del hs_sbs[g]

            def emit_act(g):
                """ACT: tT_g[:, g, :] = relu(t_ps + b2r)."""
                nc.scalar.activation(
                    tT_g[:, g, :],
                    t_pss[g],
                    RELU,
                    bias=b2r_sb,
                    scale=8.0,
                )
                del t_pss[g]

            def emit_store(g0, g1, r0):
                """Store tT_g groups [g0, g1) to dram in the grouped
                layout with ONE 2-dim DMA: src partitions 0-99 enumerate
                (i, 5w+c) i-major, exactly outG's [i, r] plane order.
                Host de-tiles the grouping.  Scalar/gpsimd rings only -
                sync is the load artery."""
                for h, ring in ((0, nc.scalar), (1, nc.gpsimd)):
                    ring.dma_start(
                        out=bass.AP(
                            outG_t,
                            50 * h * NGRP * BLK + BLK * g0,
                            [[NGRP * BLK, 50], [1, BLK * (g1 - g0)]],
                        ),
                        in_=tT_g[50 * h : 50 * h + 50, g0:g1, :],
                    )

            # ---- main loop (software-pipelined, one iteration per group) ----
            for g in range(NGRP):
                emit_group_mms(g)
                if g >= 1:
                    emit_l2(g - 1)
                emit_relu(g)
                if g >= 1:
                    emit_act(g - 1)
                if g == 11:
                    emit_store(0, 9, 0)
                elif g == 12:
                    emit_store(9, 12, 1)
            emit_l2(NGRP - 1)
            emit_act(NGRP - 1)
            emit_store(NGRP - 1, NGRP, 1)

    split_multiwaits(nc)
    return nc


def make_shards(x):
    """Per-core xT [320, PAD] fp8-e3m4 shards, +-2 col halo, zero padded."""
    xbT = np.ascontiguousarray(x.astype(F8).T)  # [320, N]
    shards = []
    for c in range(NCORES):
        s = np.zeros((D, PAD), dtype=F8)
        lo = ROWS * c - HALF
        src_lo, src_hi = max(lo, 0), min(lo + PAD, N)
        s[:, src_lo - lo : src_lo - lo + (src_hi - src_lo)] = xbT[
            :, src_lo:src_hi
        ]
        shards.append(s)
    return shards


def make_xp(xbT):
    """Need-ordered flat load regions from one core's xT [320, PAD]."""
    out = []
    for g0, g1, wdt in REGIONS:
        reg = np.zeros((128, wdt), dtype=F8)
        for g in range(g0, g1):
            ncols = 2048 if g < NGRP - 1 else BLK
            cs = 2048 * g
            k = g - g0
            c0 = 5120 * k if g < NGRP - 1 else 0
            step = 2048 if g < NGRP - 1 else BLK
            reg[:, c0 : c0 + ncols] = xbT[0:128, cs : cs + ncols]
            reg[:, c0 + step : c0 + step + ncols] = xbT[128:256, cs : cs + ncols]
            c3 = xbT[256:320, cs : cs + ncols]
            if g < NGRP - 1:
                reg[:, c0 + 4096 : c0 + 5120] = (
                    c3.reshape(64, 2, 2, BLK).transpose(2, 0, 1, 3).reshape(128, 1024)
                )
            else:
                reg[0:64, c0 + 2 * BLK : c0 + 3 * BLK] = c3
        out.append(reg)
    return np.concatenate([r.reshape(-1) for r in out])


def make_wc(W1, W2):
    """Packed bf16 stationary weights [128, 184]."""
    wc = np.zeros((128, WCEND), dtype=np.float32)
    W1T = W1.T  # [320, 10]
    wc[:, WC1:WC1 + D1] = W1T[0:128]
    wc[:, WC2:WC2 + D1] = W1T[128:256]
    wc[0:64, WC3:WC3 + D1] = W1T[256:320]
    wc[64:128, WC3 + 32 : WC3 + 32 + D1] = W1T[256:320]
    W2rep = np.tile(W2.T, (1, W))  # [10, 25]
    for i in range(4):
        wc[32 * i : 32 * i + D1, WL2 + 25 * i : WL2 + 25 * (i + 1)] = W2rep
    return np.ascontiguousarray(wc.astype(BF))


def _patch_edges(out):
    # the reference zero-pads t, not x: window slots that fall outside
    # [0, N) must be exactly zero.
    out[0, : 2 * D2] = 0.0
    out[1, :D2] = 0.0
    out[N - 2, 4 * D2 :] = 0.0
    out[N - 1, 3 * D2 :] = 0.0
    return out


def run(inputs, trace=False):
    from concourse.bass_utils import run_bass_kernel_spmd

    x = np.ascontiguousarray(np.asarray(inputs["x"], dtype=np.float32))
    W1 = np.asarray(inputs["W1"], dtype=np.float32)
    W2 = np.asarray(inputs["W2"], dtype=np.float32)
    b2 = np.ascontiguousarray(np.asarray(inputs["b2"], dtype=np.float32))
    assert x.shape == (N, D)

    WC = make_wc(W1, W2)

    if "nc" not in _NC_CACHE:
        _NC_CACHE["nc"] = build_nc()
    nc = _NC_CACHE["nc"]

    B2R = np.ascontiguousarray(np.tile(b2 * 8.0, 20).astype(np.float32))
    in_maps = [
        {"XP": make_xp(s), "WC": WC, "B2R": B2R} for s in make_shards(x)
    ]
    res = run_bass_kernel_spmd(nc, in_maps, list(range(NCORES)), trace=trace)
    cores = []
    for c in range(NCORES):
        og = np.asarray(res.results[c]["outG"]).astype(np.float32) / 8.0
        # de-tile the block grouping: [i, r, 512g+jj] -> [r, 2048g+512i+jj]
        flat = np.ascontiguousarray(
            og.reshape(GRP, 25, NGRP, BLK).transpose(1, 2, 0, 3)
        ).reshape(25, GRP * NGRP * BLK)
        core = np.empty((25, ROWS), dtype=np.float32)
        for w in range(W):  # out[5w+c, n] = t[c, n+w] = flat[5w+c, n+w]
            core[5 * w : 5 * w + D2] = flat[5 * w : 5 * w + D2, w : w + ROWS]
        cores.append(core)
    out = np.ascontiguousarray(np.concatenate(cores, axis=1).T)
    return _patch_edges(out), res


def kernel(**inputs):
    out, _ = run(inputs, trace=False)
    return out
